# revision 1
# baseline (speedup 1.0000x reference)
"""Trainium2 Bass kernel for batched self-attention with input projections.

Problem: B=8, N=2048, D=131
    Q = q @ Wq.T + bq;  K = k @ Wk.T + bk;  V = v @ Wv.T + bv
    out = softmax(Q K^T / sqrt(131)) V

One batch element per NeuronCore (8 cores, no communication).

Host prep (layout/algebra only):
  - Tokens augmented with a ones-row: X = [x^T; 1] in [132, 2048] so biases
    fold into the projection matmuls.
  - Scores: Q K^T = Xq (Wq'^T Wk'/sqrt(D)) Xk^T = Xq G Xk^T, G [132,132].
    SVD-truncate G to rank 128 (exact rank 131; error ~2e-5) so the big S
    matmul is a single K=128 contraction:  S = (Xq Aq)(Xk Ak)^T.
  - Value path: W2 [132,132] maps X -> [V | 1] (bias row + denominator
    ones-column).  SVD-truncate W2 = L R^T to rank 128 so the O-matmul
    contracts into a 128-wide latent:  O' = (P Xv L) R^T, with O'[:,131]
    the softmax denominator.  Measured end-to-end rel err ~3.8e-3 in bf16.
  - Everything bf16 (PE runs bf16 at 4x fp32); fp32 PSUM accumulation.
    |S| < 3 so softmax without max-subtraction is safe.

Per core:
  QT[e',n] = Aq^T Xq, KT[e',n] = Ak^T Xk      (2 d-chunks: 0:128, 128:132)
  VL[n,l]  = Xv^T L                            (16 j-blocks of [128,128])
  for i-half h (1024 cols), j-block (16):
      ST = KT_j^T QT[:,h]  -> exp on ACT -> E (bf16, [128,1024])
      Ohat^T[l, h] += VL_j^T-matmul with E     (PSUM accumulate over j)
  O'[i,132] = Ohat_i R^T;  out = O'[:,0:131] / O'[:,131]

All SBUF projection tensors are chunked [128,512] tiles so the attention
stream starts as soon as its first chunks are projected; a burst of junk
matmuls during the input DMA warms the PE clock gate (HAM); a post-finalize
pass drops Ldweights instructions that reload the identical weights.
"""

import numpy as np
import ml_dtypes

P = 128          # partitions / PE width
N = 2048         # tokens per core
D = 131          # embed dim
DP = 132         # embed dim + ones row
DLO = DP - P     # tail contraction rows (4)
R = 128          # truncated rank (QK interaction and V latent)
EV = 132         # final output cols (131 + denominator)
NB = N // P      # 16 token blocks
HW = 1024        # i-half width
NH = N // HW     # 2 halves
NCORES = 8

QOFF, KOFF, VOFF = 0, N, 2 * N          # column offsets in packed xall
AQOFF, AKOFF, LOFF = 0, R, 2 * R        # column offsets in packed weights

_BF16 = ml_dtypes.bfloat16


def build_nc():
    """Build the single-core Bass graph (same NEFF runs SPMD on all 8 cores)."""
    from contextlib import ExitStack

    import concourse.bacc as bacc
    import concourse.mybir as mybir
    import concourse.tile as tile
    from concourse.bass import ts

    bf = mybir.dt.bfloat16
    f32 = mybir.dt.float32
    EXP = mybir.ActivationFunctionType.Exp
    COPY = mybir.ActivationFunctionType.Copy

    nc = bacc.Bacc()
    xall = nc.declare_dram_parameter("xall", [DP, 3 * N], bf, isOutput=False)
    wpack = nc.declare_dram_parameter("wpack", [DP, 3 * R], bf, isOutput=False)
    rmat = nc.declare_dram_parameter("rmat", [R, EV], bf, isOutput=False)
    out = nc.declare_dram_parameter("out", [N, D], f32, isOutput=True)

    with tile.TileContext(nc) as tc, ExitStack() as ctx:
        const = ctx.enter_context(tc.tile_pool(name="const", bufs=1))
        xin = ctx.enter_context(tc.tile_pool(name="xin", bufs=1))
        proj = ctx.enter_context(tc.tile_pool(name="proj", bufs=1))
        epool = ctx.enter_context(tc.tile_pool(name="epool", bufs=NB + 2))
        ohs = ctx.enter_context(tc.tile_pool(name="ohs", bufs=1))
        outp = ctx.enter_context(tc.tile_pool(name="outp", bufs=2))
        warm = ctx.enter_context(tc.tile_pool(name="warm", bufs=1))
        # PSUM budget (8 banks): proj/final 2 x [128,512] = 2, scores
        # 2 x [128,1024] = 4, Ohat accumulator 1 x [128,1024] = 2.
        psp = ctx.enter_context(tc.tile_pool(name="psp", bufs=2, space="PSUM"))
        psst = ctx.enter_context(tc.tile_pool(name="psst", bufs=2, space="PSUM"))
        psoh = ctx.enter_context(tc.tile_pool(name="psoh", bufs=1, space="PSUM"))

        # ---- DMA loads.  The big inputs are split into 4 partition-range
        # chunks each: 32 descriptors per dma_start (fast issue) and the
        # chunks spread over the 8 HWDGE queues so transfers run parallel.
        wp_hi = const.tile([P, 3 * R], bf)
        nc.sync.dma_start(out=wp_hi, in_=wpack[0:P, :])
        wp_lo = const.tile([DLO, 3 * R], bf)
        nc.sync.dma_start(out=wp_lo, in_=wpack[P:DP, :])
        xall_hi = xin.tile([P, 3 * N], bf)
        xall_lo = xin.tile([DLO, 3 * N], bf)
        nc.sync.dma_start(out=xall_lo, in_=xall[P:DP, :])
        for off in (QOFF, KOFF, VOFF):
            for s in range(0, P, 32):
                nc.sync.dma_start(
                    out=xall_hi[s:s + 32, off:off + N],
                    in_=xall[s:s + 32, off:off + N],
                )
        rmat_s = const.tile([R, EV], bf)
        nc.sync.dma_start(out=rmat_s, in_=rmat[:, :])

        # ---- PE warm-up during the DMA window: dense junk matmuls flip the
        # HAM clock gate (~3.4us of sustained activity) so the projections
        # run at 2.4GHz.  No data deps -> the scheduler front-loads them.
        wsrc = warm.tile([P, 512], bf)
        nc.vector.memset(wsrc, 0)
        for w in range(10):
            pw = psst.tile([P, HW], f32, tag="pst", name="pw")
            nc.tensor.matmul(pw[:, 0:512], wsrc[:, 0:P], wsrc, start=True, stop=True)
        # second warm-up stage gated on the weights DMA so PE activity tracks
        # actual DMA progress (transfer times vary run to run)
        for w in range(8):
            pw = psst.tile([P, HW], f32, tag="pst", name="pw2")
            nc.tensor.matmul(pw[:, 0:512], wp_hi[:, 0:P], wsrc, start=True, stop=True)
        # further stages gated on the q and k input transfers: PE activity
        # tracks DMA progress so the HAM clock gate stays open into the
        # projections no matter how slow the transfers run
        for w in range(4):
            pw = psst.tile([P, HW], f32, tag="pst", name="pw3")
            nc.tensor.matmul(
                pw[:, 0:512], xall_hi[:, QOFF:QOFF + P],
                xall_hi[:, QOFF:QOFF + 512], start=True, stop=True,
            )
        for w in range(4):
            pw = psst.tile([P, HW], f32, tag="pst", name="pw4")
            nc.tensor.matmul(
                pw[:, 0:512], xall_hi[:, KOFF:KOFF + P],
                xall_hi[:, KOFF:KOFF + 512], start=True, stop=True,
            )

        # ---- projections, chunked so attention can start early.
        # Order: the h=0 attention stream needs QT chunks 0-1 and all KT
        # chunks, so project those first; QT 2-3 (h=1) and VL (Ohat) follow.
        qts = [proj.tile([P, 512], bf, tag=f"qt{c}", name=f"qt{c}") for c in range(4)]
        kts = [proj.tile([P, 512], bf, tag=f"kt{c}", name=f"kt{c}") for c in range(4)]
        vls = [proj.tile([P, 512], bf, tag=f"vl{c}", name=f"vl{c}") for c in range(4)]

        def qk_chunk(dst, woff, xoff, c):
            pp = psp.tile([P, 512], f32, tag="pp", name="pp")
            nc.tensor.matmul(
                pp,
                wp_hi[:, woff:woff + R],
                xall_hi[:, xoff + c * 512: xoff + (c + 1) * 512],
                start=True,
                stop=False,
            )
            nc.tensor.matmul(
                pp,
                wp_lo[:, woff:woff + R],
                xall_lo[:, xoff + c * 512: xoff + (c + 1) * 512],
                start=False,
                stop=True,
            )
            nc.vector.tensor_copy(dst, pp)

        def vl_group(g):
            pv = psp.tile([P, 512], f32, tag="pp", name="pv")
            for t in range(4):
                j = 4 * g + t
                nc.tensor.matmul(
                    pv[:, ts(t, P)],
                    xall_hi[:, VOFF + j * P: VOFF + (j + 1) * P],
                    wp_hi[:, LOFF:LOFF + R],
                    start=True,
                    stop=False,
                )
                nc.tensor.matmul(
                    pv[:, ts(t, P)],
                    xall_lo[:, VOFF + j * P: VOFF + (j + 1) * P],
                    wp_lo[:, LOFF:LOFF + R],
                    start=False,
                    stop=True,
                )
            nc.vector.tensor_copy(vls[g], pv)

        def s_exp(h, j, es):
            pst = psst.tile([P, HW], f32, tag="pst", name="pst")
            for c in range(2):
                nc.tensor.matmul(
                    pst[:, ts(c, 512)],
                    kts[j // 4][:, ts(j % 4, P)],
                    qts[2 * h + c],
                    start=True,
                    stop=True,
                )
            ej = epool.tile([P, HW], bf, tag="e", name="ej")
            nc.scalar.activation(ej, pst, EXP)
            es.append(ej)

        # Interleave: each KT chunk immediately feeds its 4 S/exp waves so
        # the ACT exp stream reaches steady cadence while the remaining
        # projections fill PE slack.
        es0 = []
        qk_chunk(qts[0], AQOFF, QOFF, 0)
        qk_chunk(qts[1], AQOFF, QOFF, 1)
        for c in range(4):
            qk_chunk(kts[c], AKOFF, KOFF, c)
            for j in range(4 * c, 4 * c + 4):
                s_exp(0, j, es0)
        for g in range(4):
            vl_group(g)
        qk_chunk(qts[2], AQOFF, QOFF, 2)
        qk_chunk(qts[3], AQOFF, QOFF, 3)

        # ---- attention + per-half finalization ----
        for h in range(NH):
            if h == 0:
                es = es0
            else:
                es = []
                for j in range(NB):
                    s_exp(h, j, es)
            poh = psoh.tile([P, HW], f32, tag="poh", name="poh")
            for j in range(NB):
                for c in range(2):
                    nc.tensor.matmul(
                        poh[:, ts(c, 512)],
                        vls[j // 4][:, ts(j % 4, P)],
                        es[j][:, ts(c, 512)],
                        start=(j == 0),
                        stop=(j == NB - 1),
                    )
            ohat = ohs.tile([P, HW], bf, tag=f"oh{h}", name=f"oh{h}")
            if h == 0:
                nc.vector.tensor_copy(ohat, poh)
            else:
                # ACT is free once the exp stream ends; split the copy
                nc.scalar.activation(ohat[:, 0:512], poh[:, 0:512], COPY)
                nc.vector.tensor_copy(ohat[:, 512:HW], poh[:, 512:HW])

            # finalize this half's 8 i-blocks (4 output groups of 2);
            # h=0's work overlaps h=1's exp stream.
            for g in range(4 * h, 4 * h + 4):
                stage = outp.tile([P, 2, D], f32, tag="stage", name="stage")
                for t in range(2):
                    i = 2 * g + t
                    po = psp.tile([P, EV], f32, tag="pp", name="po")
                    nc.tensor.matmul(
                        po, ohat[:, ts(i % 8, P)], rmat_s, start=True, stop=True
                    )
                    rec = outp.tile([P, 1], f32, tag="rec", name="rec")
                    nc.vector.reciprocal(rec, po[:, D:D + 1])
                    # alternate engines so consecutive i-blocks normalize in
                    # parallel instead of chaining on one engine
                    if (h == 0) == (t == 0):
                        nc.vector.tensor_scalar_mul(stage[:, t, :], po[:, 0:D], rec)
                    else:
                        nc.scalar.activation(
                            stage[:, t, :], po[:, 0:D], COPY, scale=rec
                        )
                nc.sync.dma_start(
                    out=out[g * 256:(g + 1) * 256, :].rearrange(
                        "(t p) e -> p t e", p=P
                    ),
                    in_=stage,
                )

    return nc


def dedup_ldweights(nc):
    """Drop Ldweights instructions that reload the exact weights already in
    the PE array (same AP, nothing clobbering in between).  The PE keeps the
    stationary operand across matmuls, so a back-to-back identical reload is
    pure dispatch overhead (~107ns each).  Only sync-free Ldweights are
    dropped so semaphore ordering is untouched."""
    dropped = 0
    for f in nc.m.functions:
        for blk in f.blocks:
            insts = list(blk.instructions)
            kept = []
            last_key = None
            for ins in insts:
                tname = type(ins).__name__
                if "PE" in str(getattr(ins, "engine", "")):
                    if tname == "InstLdweights":
                        ap = ins.ins[0]
                        key = (
                            ap.memref,
                            ap.offset,
                            str(ap.ap),
                            str(ap.dtype),
                            str(getattr(ins, "is_transpose", None)),
                        )
                        si = ins.sync_info
                        no_sync = si is None or (
                            len(si.on_wait) == 0 and len(si.on_update) == 0
                        )
                        if key == last_key and no_sync:
                            dropped += 1
                            continue
                        last_key = key
                    elif tname not in (
                        "InstMatmult",
                        "InstEventSemaphore",
                        "InstNoOp",
                        "InstDrain",
                    ):
                        last_key = None
                kept.append(ins)
            if len(kept) != len(insts):
                blk.instructions = kept
    return dropped


def prep_host(query, key, value, Wq, bq, Wk, bk, Wv, bv):
    """Host-side layout/algebra prep. Returns per-core input maps."""
    s = np.sqrt(np.float64(D))
    Wqp = np.concatenate([Wq, bq[:, None]], axis=1)  # [131, 132]
    Wkp = np.concatenate([Wk, bk[:, None]], axis=1)
    G = (Wqp.astype(np.float64).T @ Wkp.astype(np.float64)) / s  # [132, 132]
    U, S, Vt = np.linalg.svd(G)
    Aq = (U[:, :R] * np.sqrt(S[:R])).astype(np.float32)  # [132, 128]
    Ak = (Vt[:R, :].T * np.sqrt(S[:R])).astype(np.float32)

    W2 = np.zeros((DP, EV), np.float64)  # maps X -> [V | 1]
    W2[:D, :D] = Wv.T
    W2[D, :D] = bv
    W2[D, D] = 1.0
    U2, S2, V2t = np.linalg.svd(W2)
    L = (U2[:, :R] * np.sqrt(S2[:R])).astype(np.float32)  # [132, 128]
    Rm = (V2t[:R, :].T * np.sqrt(S2[:R])).astype(np.float32)  # [132, 128]

    wpack = np.concatenate([Aq, Ak, L], axis=1)  # [132, 384]
    wpack16 = np.ascontiguousarray(wpack.astype(_BF16))
    rmat16 = np.ascontiguousarray(Rm.T.astype(_BF16))  # [128, 132]

    ones_row = np.ones((1, N), np.float32)
    in_maps = []
    for c in range(NCORES):
        xs = [np.concatenate([x.T, ones_row], axis=0)
              for x in (query[c], key[c], value[c])]
        xallc = np.concatenate(xs, axis=1)  # [132, 6144]
        in_maps.append({
            "xall": np.ascontiguousarray(xallc.astype(_BF16)),
            "wpack": wpack16,
            "rmat": rmat16,
        })
    return in_maps


_NC_CACHE = {}


def _get_nc():
    if "nc" not in _NC_CACHE:
        nc = build_nc()
        if not nc.is_finalized():
            nc.finalize()  # Bacc.finalize runs the wait-split/EVSEM passes
        dedup_ldweights(nc)
        _NC_CACHE["nc"] = nc
    return _NC_CACHE["nc"]


def run_on_cores(in_maps, trace=False, **kw):
    from concourse.bass_utils import run_bass_kernel_spmd

    nc = _get_nc()
    return run_bass_kernel_spmd(nc, in_maps, core_ids=list(range(NCORES)),
                                trace=trace, **kw)


def kernel(query, key, value, Wq, bq, Wk, bk, Wv, bv):
    in_maps = prep_host(query, key, value, Wq, bq, Wk, bk, Wv, bv)
    res = run_on_cores(in_maps)
    return np.stack([np.asarray(res.results[c]["out"]) for c in range(NCORES)])



# revision 4
# speedup vs baseline: 1.1076x; 1.1076x over previous
"""Trainium2 Bass kernel for batched self-attention with input projections.

Problem: B=8, N=2048, D=131
    Q = q @ Wq.T + bq;  K = k @ Wk.T + bk;  V = v @ Wv.T + bv
    out = softmax(Q K^T / sqrt(131)) V

One batch element per NeuronCore (8 cores, no communication).

Host prep (layout/algebra only):
  - Tokens augmented with a ones-row: X = [x^T; 1] in [132, 2048] so biases
    fold into the projection matmuls.
  - Scores: Q K^T = Xq (Wq'^T Wk'/sqrt(D)) Xk^T = Xq G Xk^T, G [132,132].
    SVD-truncate G to rank 128 (exact rank 131; error ~2e-5) so the big S
    matmul is a single K=128 contraction:  S = (Xq Aq)(Xk Ak)^T.
  - Value path: W2 [132,132] maps X -> [V | 1] (bias row + denominator
    ones-column).  SVD-truncate W2 = L R^T to rank 128 so the O-matmul
    contracts into a 128-wide latent:  O' = (P Xv L) R^T, with O'[:,131]
    the softmax denominator.
  - Projections/S in bf16 (fp32 PSUM accumulation); the O path (exp
    weights E and the value latent VL) in fp8e4m3 so the O accumulation
    runs in DoubleRow perf mode: each matmul contracts TWO 128-token
    k-tiles at once (lhsT [128,2,128], rhs [128,2,1024]), halving the PE
    time of the biggest accumulation.  Measured rel err ~1.05e-2 (numpy
    bit-accurate sim of the dtype pipeline).  |S| < 3 so softmax without
    max-subtraction is safe.

Per core:
  QT[e',n] = Aq^T Xq, KT[e',n] = Ak^T Xk      (hi[128]+lo[4] d-chunks)
  VL[n,l]  = Xv^T L -> fp8 pair tiles [128,2,128] (j, j+1)
  for i-half h (1024 cols), j-block (16):
      ST = KT_j^T QT_h  (one [128,1024] matmul) -> exp on ACT -> fp8 E
      pair tiles [128,2,1024]; every 2 j's one DoubleRow matmul
      accumulates Ohat^T[l, h] in PSUM.
  O'[i,132] = Ohat_i R^T;  out = O'[:,0:131] / O'[:,131]

ACT does exp only (the activation table preloads via a dummy exp during
the DMA window); DVE does all PSUM->SBUF copies and the normalize; a
burst of junk matmuls during the input DMA warms the PE clock gate; a
post-finalize pass drops Ldweights instructions that reload identical
weights.
"""

import numpy as np
import ml_dtypes

P = 128          # partitions / PE width
N = 2048         # tokens per core
D = 131          # embed dim
DP = 132         # embed dim + ones row
DLO = DP - P     # tail contraction rows (4)
R = 128          # truncated rank (QK interaction and V latent)
EV = 132         # final output cols (131 + denominator)
NB = N // P      # 16 token blocks
NPAIR = NB // 2  # 8 j-block pairs (DoubleRow granule)
HW = 1024        # i-half width
NH = N // HW     # 2 halves
NCORES = 8

QOFF, KOFF, VOFF = 0, N, 2 * N          # column offsets in packed xall
AQOFF, AKOFF, LOFF = 0, R, 2 * R        # column offsets in packed weights

_BF16 = ml_dtypes.bfloat16


def build_nc():
    """Build the single-core Bass graph (same NEFF runs SPMD on all 8 cores)."""
    from contextlib import ExitStack

    import concourse.bacc as bacc
    import concourse.mybir as mybir
    import concourse.tile as tile
    from concourse.bass import ts

    bf = mybir.dt.bfloat16
    f8 = mybir.dt.float8e4
    f32 = mybir.dt.float32
    EXP = mybir.ActivationFunctionType.Exp
    COPY = mybir.ActivationFunctionType.Copy
    DR = mybir.MatmulPerfMode.DoubleRow

    nc = bacc.Bacc()
    xall = nc.declare_dram_parameter("xall", [DP, 3 * N], bf, isOutput=False)
    wpack = nc.declare_dram_parameter("wpack", [DP, 3 * R], bf, isOutput=False)
    rmat = nc.declare_dram_parameter("rmat", [R, EV], bf, isOutput=False)
    out = nc.declare_dram_parameter("out", [N, D], f32, isOutput=True)

    with tile.TileContext(nc) as tc, ExitStack() as ctx:
        const = ctx.enter_context(tc.tile_pool(name="const", bufs=1))
        xin = ctx.enter_context(tc.tile_pool(name="xin", bufs=1))
        proj = ctx.enter_context(tc.tile_pool(name="proj", bufs=1))
        vpool = ctx.enter_context(tc.tile_pool(name="vpool", bufs=1))
        epool = ctx.enter_context(tc.tile_pool(name="epool", bufs=6))
        ohs = ctx.enter_context(tc.tile_pool(name="ohs", bufs=1))
        outp = ctx.enter_context(tc.tile_pool(name="outp", bufs=2))
        warm = ctx.enter_context(tc.tile_pool(name="warm", bufs=1))
        # PSUM budget (8 banks): proj/final 2 x [128,512] = 2, scores
        # 2 x [128,1024] = 4, Ohat accumulator 1 x [128,1024] = 2.
        psp = ctx.enter_context(tc.tile_pool(name="psp", bufs=2, space="PSUM"))
        psst = ctx.enter_context(tc.tile_pool(name="psst", bufs=2, space="PSUM"))
        psoh = ctx.enter_context(tc.tile_pool(name="psoh", bufs=1, space="PSUM"))

        # ---- DMA loads.  Column-chunked [128, 1024] transfers (one 2D
        # descriptor each, 2KB per partition line) ordered so the h=0
        # attention stream's inputs land first: Q cols 0:1024, then K, V,
        # then Q cols 1024:2048 (h=1).
        wp_hi = const.tile([P, 3 * R], bf)
        nc.sync.dma_start(out=wp_hi, in_=wpack[0:P, :])
        wp_lo = const.tile([DLO, 3 * R], bf)
        nc.sync.dma_start(out=wp_lo, in_=wpack[P:DP, :])
        xall_hi = xin.tile([P, 3 * N], bf)
        xall_lo = xin.tile([DLO, 3 * N], bf)
        nc.sync.dma_start(out=xall_lo, in_=xall[P:DP, :])
        for lo, hi in (
            (QOFF, QOFF + HW),           # Q half 0
            (KOFF, KOFF + HW),           # K chunks 0-1
            (KOFF + HW, KOFF + N),       # K chunks 2-3
            (VOFF, VOFF + HW),           # V blocks 0-7
            (VOFF + HW, VOFF + N),       # V blocks 8-15
            (QOFF + HW, QOFF + N),       # Q half 1
        ):
            nc.sync.dma_start(
                out=xall_hi[:, lo:hi], in_=xall[0:P, lo:hi]
            )
        rmat_s = const.tile([R, EV], bf)
        nc.sync.dma_start(out=rmat_s, in_=rmat[:, :])

        # ---- warm-up during the DMA window: the dummy exp pulls the ACT
        # table load (~1.3us) off the exp stream; junk matmuls keep the PE
        # clock gate (HAM) open so projections run at full clock.
        wsrc = warm.tile([P, 512], bf)
        nc.vector.memset(wsrc, 0)
        wdum = warm.tile([P, 4], bf)
        nc.scalar.activation(wdum, wsrc[:, 0:4], EXP)
        for w in range(6):
            pw = psst.tile([P, HW], f32, tag="pst", name="pw")
            nc.tensor.matmul(pw[:, 0:512], wsrc[:, 0:P], wsrc, start=True, stop=True)
        # stages gated on the weights / q / k transfers so PE activity
        # tracks DMA progress (transfer times vary run to run)
        for w in range(4):
            pw = psst.tile([P, HW], f32, tag="pst", name="pw2")
            nc.tensor.matmul(pw[:, 0:512], wp_hi[:, 0:P], wsrc, start=True, stop=True)
        for w in range(3):
            pw = psst.tile([P, HW], f32, tag="pst", name="pw3")
            nc.tensor.matmul(
                pw[:, 0:512], xall_hi[:, QOFF:QOFF + P],
                xall_hi[:, QOFF:QOFF + 512], start=True, stop=True,
            )
        for w in range(3):
            pw = psst.tile([P, HW], f32, tag="pst", name="pw4")
            nc.tensor.matmul(
                pw[:, 0:512], xall_hi[:, KOFF:KOFF + P],
                xall_hi[:, KOFF:KOFF + 512], start=True, stop=True,
            )

        # ---- projection tiles.  QT merged per half so each S_j is ONE
        # [128,1024] matmul; KT chunked [128,512]; VL as fp8 pair tiles.
        qth = [proj.tile([P, HW], bf, tag=f"qh{h}", name=f"qh{h}") for h in range(NH)]
        kts = [proj.tile([P, 512], bf, tag=f"kt{c}", name=f"kt{c}") for c in range(4)]
        vps = [vpool.tile([P, 2, P], f8, tag=f"vp{g}", name=f"vp{g}")
               for g in range(NPAIR)]

        def qk_chunk2(specs):
            """Project 1-2 chunks with hi-matmuls first so the identical
            stationary Ldweights dedup back-to-back."""
            pps = []
            for dst, woff, xoff, c in specs:
                pp = psp.tile([P, 512], f32, tag="pp", name="pp")
                nc.tensor.matmul(
                    pp,
                    wp_hi[:, woff:woff + R],
                    xall_hi[:, xoff + c * 512: xoff + (c + 1) * 512],
                    start=True,
                    stop=False,
                )
                pps.append(pp)
            for (dst, woff, xoff, c), pp in zip(specs, pps):
                nc.tensor.matmul(
                    pp,
                    wp_lo[:, woff:woff + R],
                    xall_lo[:, xoff + c * 512: xoff + (c + 1) * 512],
                    start=False,
                    stop=True,
                )
            for (dst, woff, xoff, c), pp in zip(specs, pps):
                nc.vector.tensor_copy(dst, pp)

        def vl_group(g4):
            """Project VL for j = 4*g4 .. 4*g4+3 into fp8 pair tiles."""
            pv = psp.tile([P, 512], f32, tag="pp", name="pv")
            for t in range(4):
                j = 4 * g4 + t
                nc.tensor.matmul(
                    pv[:, ts(t, P)],
                    xall_hi[:, VOFF + j * P: VOFF + (j + 1) * P],
                    wp_hi[:, LOFF:LOFF + R],
                    start=True,
                    stop=False,
                )
                nc.tensor.matmul(
                    pv[:, ts(t, P)],
                    xall_lo[:, VOFF + j * P: VOFF + (j + 1) * P],
                    wp_lo[:, LOFF:LOFF + R],
                    start=False,
                    stop=True,
                )
            for t in range(2):
                g = 2 * g4 + t
                nc.vector.tensor_copy(vps[g][:, 0, :], pv[:, ts(2 * t, P)])
                nc.vector.tensor_copy(vps[g][:, 1, :], pv[:, ts(2 * t + 1, P)])

        def s_exp(h, j, edst):
            """S^T_j for half h (512-col matmuls: PSUM-bank limit), exp (fp8)."""
            pst = psst.tile([P, HW], f32, tag="pst", name="pst")
            for c in range(2):
                nc.tensor.matmul(
                    pst[:, ts(c, 512)],
                    kts[j // 4][:, ts(j % 4, P)],
                    qth[h][:, ts(c, 512)],
                    start=True,
                    stop=True,
                )
            nc.scalar.activation(edst, pst, EXP)

        def o_pair(poh, g, ep):
            """DoubleRow fp8 matmuls: contract j-blocks 2g and 2g+1 at once."""
            for c in range(2):
                nc.tensor.matmul(
                    poh[:, ts(c, 512)],
                    vps[g],
                    ep[:, :, ts(c, 512)],
                    start=(g == 0),
                    stop=(g == NPAIR - 1),
                    perf_mode=DR,
                )

        def finalize_group(h, g, ohat):
            """Two i-blocks -> O' = Ohat R^T, normalize, DMA out."""
            stage = outp.tile([P, 2, D], f32, tag="stage", name="stage")
            for t in range(2):
                i = 2 * g + t
                po = psp.tile([P, EV], f32, tag="pp", name="po")
                nc.tensor.matmul(
                    po, ohat[:, ts(i % 8, P)], rmat_s, start=True, stop=True
                )
                rec = outp.tile([P, 1], f32, tag="rec", name="rec")
                nc.vector.reciprocal(rec, po[:, D:D + 1])
                nc.vector.tensor_scalar_mul(stage[:, t, :], po[:, 0:D], rec)
            nc.sync.dma_start(
                out=out[g * 256:(g + 1) * 256, :].rearrange(
                    "(t p) e -> p t e", p=P
                ),
                in_=stage,
            )

        # ---- h=0 stream: project what each j needs just in time, start
        # the exp stream as early as possible, trail it with the DoubleRow
        # O accumulation; VL groups fill PE slack between S matmuls.
        qk_chunk2([
            (qth[0][:, 0:512], AQOFF, QOFF, 0),
            (qth[0][:, 512:HW], AQOFF, QOFF, 1),
        ])
        qk_chunk2([(kts[0], AKOFF, KOFF, 0)])

        poh0 = psoh.tile([P, HW], f32, tag="poh", name="poh0")
        eps0 = []

        def h0_step(j):
            if j % 2 == 0:
                ep = epool.tile([P, 2, HW], f8, tag="e", name=f"e0_{j // 2}")
                eps0.append(ep)
            s_exp(0, j, eps0[j // 2][:, j % 2, :])

        h0_step(0)
        h0_step(1)
        qk_chunk2([(kts[1], AKOFF, KOFF, 1), (kts[2], AKOFF, KOFF, 2)])
        h0_step(2)
        h0_step(3)
        vl_group(0)
        h0_step(4)
        o_pair(poh0, 0, eps0[0])
        h0_step(5)
        vl_group(1)
        h0_step(6)
        o_pair(poh0, 1, eps0[1])
        h0_step(7)
        qk_chunk2([(kts[3], AKOFF, KOFF, 3)])
        h0_step(8)
        o_pair(poh0, 2, eps0[2])
        h0_step(9)
        vl_group(2)
        h0_step(10)
        o_pair(poh0, 3, eps0[3])
        h0_step(11)
        vl_group(3)
        h0_step(12)
        o_pair(poh0, 4, eps0[4])
        h0_step(13)
        qk_chunk2([
            (qth[1][:, 0:512], AQOFF, QOFF, 2),
            (qth[1][:, 512:HW], AQOFF, QOFF, 3),
        ])
        h0_step(14)
        o_pair(poh0, 5, eps0[5])
        h0_step(15)
        o_pair(poh0, 6, eps0[6])
        o_pair(poh0, 7, eps0[7])
        ohat0 = ohs.tile([P, HW], bf, tag="oh0", name="oh0")
        nc.vector.tensor_copy(ohat0, poh0)

        # ---- h=1 stream with h=0 finalization interleaved.
        poh1 = psoh.tile([P, HW], f32, tag="poh", name="poh1")
        eps1 = []

        def h1_step(j):
            if j % 2 == 0:
                ep = epool.tile([P, 2, HW], f8, tag="e", name=f"e1_{j // 2}")
                eps1.append(ep)
            s_exp(1, j, eps1[j // 2][:, j % 2, :])

        for j in range(NB):
            h1_step(j)
            if j % 2 == 1:
                g = j // 2
                o_pair(poh1, g, eps1[g])
                if g >= 4:
                    # h0 finals ride the late-h1 exp-wait slack on PE
                    finalize_group(0, g - 4, ohat0)
        ohat1 = ohs.tile([P, HW], bf, tag="oh1", name="oh1")
        # ACT is free once the exp stream ends; split the copy
        nc.scalar.activation(ohat1[:, 0:512], poh1[:, 0:512], COPY)
        nc.vector.tensor_copy(ohat1[:, 512:HW], poh1[:, 512:HW])
        for g in range(4):
            finalize_group(1, 4 + g, ohat1)

    return nc


def dedup_ldweights(nc):
    """Drop Ldweights instructions that reload the exact weights already in
    the PE array (same AP, nothing clobbering in between).  The PE keeps the
    stationary operand across matmuls, so a back-to-back identical reload is
    pure dispatch overhead (~107ns each).  Only sync-free Ldweights are
    dropped so semaphore ordering is untouched."""
    dropped = 0
    for f in nc.m.functions:
        for blk in f.blocks:
            insts = list(blk.instructions)
            kept = []
            last_key = None
            for ins in insts:
                tname = type(ins).__name__
                if "PE" in str(getattr(ins, "engine", "")):
                    if tname == "InstLdweights":
                        ap = ins.ins[0]
                        key = (
                            ap.memref,
                            ap.offset,
                            str(ap.ap),
                            str(ap.dtype),
                            str(getattr(ins, "is_transpose", None)),
                        )
                        si = ins.sync_info
                        no_sync = si is None or (
                            len(si.on_wait) == 0 and len(si.on_update) == 0
                        )
                        if key == last_key and no_sync:
                            dropped += 1
                            continue
                        last_key = key
                    elif tname not in (
                        "InstMatmult",
                        "InstEventSemaphore",
                        "InstNoOp",
                        "InstDrain",
                    ):
                        last_key = None
                kept.append(ins)
            if len(kept) != len(insts):
                blk.instructions = kept
    return dropped


def prep_host(query, key, value, Wq, bq, Wk, bk, Wv, bv):
    """Host-side layout/algebra prep. Returns per-core input maps."""
    s = np.sqrt(np.float64(D))
    Wqp = np.concatenate([Wq, bq[:, None]], axis=1)  # [131, 132]
    Wkp = np.concatenate([Wk, bk[:, None]], axis=1)
    G = (Wqp.astype(np.float64).T @ Wkp.astype(np.float64)) / s  # [132, 132]
    U, S, Vt = np.linalg.svd(G)
    Aq = (U[:, :R] * np.sqrt(S[:R])).astype(np.float32)  # [132, 128]
    Ak = (Vt[:R, :].T * np.sqrt(S[:R])).astype(np.float32)

    W2 = np.zeros((DP, EV), np.float64)  # maps X -> [V | 1]
    W2[:D, :D] = Wv.T
    W2[D, :D] = bv
    W2[D, D] = 1.0
    U2, S2, V2t = np.linalg.svd(W2)
    L = (U2[:, :R] * np.sqrt(S2[:R])).astype(np.float32)  # [132, 128]
    Rm = (V2t[:R, :].T * np.sqrt(S2[:R])).astype(np.float32)  # [132, 128]

    wpack = np.concatenate([Aq, Ak, L], axis=1)  # [132, 384]
    wpack16 = np.ascontiguousarray(wpack.astype(_BF16))
    rmat16 = np.ascontiguousarray(Rm.T.astype(_BF16))  # [128, 132]

    ones_row = np.ones((1, N), np.float32)
    in_maps = []
    for c in range(NCORES):
        xs = [np.concatenate([x.T, ones_row], axis=0)
              for x in (query[c], key[c], value[c])]
        xallc = np.concatenate(xs, axis=1)  # [132, 6144]
        in_maps.append({
            "xall": np.ascontiguousarray(xallc.astype(_BF16)),
            "wpack": wpack16,
            "rmat": rmat16,
        })
    return in_maps


_NC_CACHE = {}


def _get_nc():
    if "nc" not in _NC_CACHE:
        nc = build_nc()
        if not nc.is_finalized():
            nc.finalize()  # Bacc.finalize runs the wait-split/EVSEM passes
        dedup_ldweights(nc)
        _NC_CACHE["nc"] = nc
    return _NC_CACHE["nc"]


def run_on_cores(in_maps, trace=False, **kw):
    from concourse.bass_utils import run_bass_kernel_spmd

    nc = _get_nc()
    return run_bass_kernel_spmd(nc, in_maps, core_ids=list(range(NCORES)),
                                trace=trace, **kw)


def kernel(query, key, value, Wq, bq, Wk, bk, Wv, bv):
    in_maps = prep_host(query, key, value, Wq, bq, Wk, bk, Wv, bv)
    res = run_on_cores(in_maps)
    return np.stack([np.asarray(res.results[c]["out"]) for c in range(NCORES)])
